# revision 7
# baseline (speedup 1.0000x reference)
"""Trainium2 Bass kernel for the BDH-style sparse-attention model.

Model (per reference): L=6 layers over T=1024 tokens, D=256, H=4 heads,
N=32768 neurons (NH=8192 per head), strict-causal unnormalized linear
attention with RoPE over the neuron dim, gated wide projection, encoder
contraction with residual layernorms, final vocab readout.

Sharding (8 NeuronCores): tensor-parallel over (head, neuron): core c owns
head h=c//2 and half of that head's neurons (4096), chosen as a contiguous
slice of the rope *pair* space so rotary stays core-local:
  pair p=c%2 owns head-cols [p*2048,(p+1)*2048) and [4096+p*2048, ...+2048).
Per layer:
  x   = relu(v @ Wx_c)            local [T, 4096]  (PE)
  xr  = rope(x)                   local            (DVE, host tables)
  G   = xr xr^T strict-upper in (s,t) layout       (PE, trapezoid blocks)
  a   = S^T-contract with v       partial over the neuron shard
        -> pairwise AllReduce within each head's 2 cores
  y   = relu(ln(a) @ Wy_c) * x    local (x recomputed for the gate)
  e   = y @ Enc_c                 partial -> AllReduce over all 8 cores
  v   = ln(v + ln(e))             replicated
Output: v @ readout on every core (core 0's copy returned).

All matmuls run in bf16 (fp32 PSUM accumulation); validated vs the fp32
reference at ~1.1e-2 max-rel (gate 2e-2).
"""

import numpy as np
import ml_dtypes

import concourse.bass as bass
import concourse.mybir as mybir
import concourse.tile as tile
from concourse import bacc
from concourse.bass_utils import run_bass_kernel_spmd

AF = mybir.ActivationFunctionType
ALU = mybir.AluOpType
F32 = mybir.dt.float32
BF16 = mybir.dt.bfloat16

NCORES = 8
D = 256
H = 4
L = 6
N = 32768
NH = N // H          # 8192
NLOC = NH // 2       # 4096 per-core neurons
HALF = NLOC // 2     # 2048 rope pairs per core
T = 1024
VOCAB = 256
ROPE_BASE = 10000.0
NCH = NLOC // 128    # 32 chunks of 128 neurons
NPAIR = NCH // 2     # 16 rope chunk pairs
TCN = 2              # t-chunks
TCW = T // TCN       # 512
TT = T // 128        # 8 global t-tiles

REPLICA_PAIRS = [[0, 1], [2, 3], [4, 5], [6, 7]]
REPLICA_ALL = [list(range(NCORES))]


def build(nlayers: int = L):
    nc = bacc.Bacc(
        "TRN2", target_bir_lowering=False, debug=False,
        enable_asserts=False, num_devices=NCORES,
    )

    # ---- DRAM I/O ----
    wx_d = nc.dram_tensor("wx", [128, 2, NLOC], BF16, kind="ExternalInput")
    wy_d = nc.dram_tensor("wy", [128, 2, NLOC], BF16, kind="ExternalInput")
    enc_d = nc.dram_tensor("enc", [NCH, 128, D], BF16, kind="ExternalInput")
    cs_d = nc.dram_tensor("cs", [NPAIR, TCN, 128, TCW], BF16, kind="ExternalInput")
    sn_d = nc.dram_tensor("sn", [NPAIR, TCN, 128, TCW], BF16, kind="ExternalInput")
    ro_d = nc.dram_tensor("ro", [128, 2, VOCAB], BF16, kind="ExternalInput")
    v0b_d = nc.dram_tensor("v0b", [128, TT, D], BF16, kind="ExternalInput")
    v0t_d = nc.dram_tensor("v0t", [128, 2, T], BF16, kind="ExternalInput")
    mask_d = nc.dram_tensor("maskd", [128, 128], F32, kind="ExternalInput")
    ident_d = nc.dram_tensor("ident", [128, 128], BF16, kind="ExternalInput")
    out_d = nc.dram_tensor("out", [T, VOCAB], F32, kind="ExternalOutput")

    cs_ap = cs_d.ap()
    sn_ap = sn_d.ap()
    enc_ap = enc_d.ap()

    with tile.TileContext(nc) as tc:
        with (
            tc.tile_pool(name="pers", bufs=1) as pers,
            tc.tile_pool(name="chbf", bufs=2) as chbf,
            tc.tile_pool(name="trig", bufs=3) as trig,
            tc.tile_pool(name="encp", bufs=4) as encp,
            tc.tile_pool(name="s32", bufs=1) as s32,
            tc.tile_pool(name="sbf", bufs=2) as sbf,
            tc.tile_pool(name="stat", bufs=2) as statp,
            tc.tile_pool(name="pxy", bufs=2, space="PSUM") as pxy_pool,
            tc.tile_pool(name="pg", bufs=2, space="PSUM") as pg_pool,
            tc.tile_pool(name="pe", bufs=2, space="PSUM") as pe_pool,
            tc.tile_pool(name="pa", bufs=1, space="PSUM") as pa_pool,
            tc.tile_pool(name="pt", bufs=1, space="PSUM") as pt_pool,
            tc.tile_pool(name="dram", bufs=2, space="DRAM") as dram,
        ):
            # ---- persistent SBUF ----
            wx = pers.tile([128, 2, NLOC], BF16, tag="wx")
            wy = pers.tile([128, 2, NLOC], BF16, tag="wy")
            ro = pers.tile([128, 2, VOCAB], BF16, tag="ro")
            maskd = pers.tile([128, 128], F32, tag="maskd")
            ident = pers.tile([128, 128], BF16, tag="ident")
            xrT = pers.tile([128, NCH, T], BF16, tag="xrT")
            S = pers.tile([128, TT, T], BF16, tag="S")
            vbf0 = pers.tile([128, TT, D], BF16, tag="vbf0")
            vbf1 = pers.tile([128, TT, D], BF16, tag="vbf1")
            vT0 = pers.tile([128, 2, T], BF16, tag="vT0")
            vT1 = pers.tile([128, 2, T], BF16, tag="vT1")
            vbf = [vbf0, vbf1]
            vT = [vT0, vT1]
            eps = pers.tile([128, 1], F32, tag="eps")
            nc.gpsimd.memset(eps[:], 1e-5)

            nc.sync.dma_start(wx[:], wx_d[:])
            nc.sync.dma_start(wy[:], wy_d[:])
            nc.sync.dma_start(ro[:], ro_d[:])
            nc.sync.dma_start(maskd[:], mask_d[:])
            nc.sync.dma_start(ident[:], ident_d[:])
            nc.sync.dma_start(vbf[0][:], v0b_d[:])
            nc.sync.dma_start(vT[0][:], v0t_d[:])

            def x_chunk_mm(vt_cur, c, tci, relu_dst):
                """relu_dst = relu(Wx[:, :, chunk c] contracted with vT)."""
                ps = pxy_pool.tile([128, TCW], F32, tag="pxy", name=f"px_{c}_{tci}")
                for dc in range(2):
                    nc.tensor.matmul(
                        ps[:], wx[:, dc, c * 128:(c + 1) * 128],
                        vt_cur[:, dc, tci * TCW:(tci + 1) * TCW],
                        start=(dc == 0), stop=(dc == 1),
                    )
                nc.scalar.activation(relu_dst[:], ps[:], AF.Relu)

            def ln_stats(src, nt, tag):
                """src [128, nt, D] -> (rstd, -mean*rstd) each [128, nt]."""
                ssum = statp.tile([128, nt], F32, tag=f"ssum{tag}", bufs=2)
                qsum = statp.tile([128, nt], F32, tag=f"qsum{tag}", bufs=2)
                sq = s32.tile([128, nt, D], F32, tag="sqscratch", bufs=1)
                nc.vector.tensor_reduce(ssum[:], src[:], mybir.AxisListType.X, ALU.add)
                nc.scalar.activation(sq[:], src[:], AF.Square)
                nc.vector.tensor_reduce(qsum[:], sq[:], mybir.AxisListType.X, ALU.add)
                m = statp.tile([128, nt], F32, tag=f"m{tag}", bufs=2)
                var = statp.tile([128, nt], F32, tag=f"var{tag}", bufs=2)
                std = statp.tile([128, nt], F32, tag=f"std{tag}", bufs=2)
                rstd = statp.tile([128, nt], F32, tag=f"rstd{tag}", bufs=2)
                nmr = statp.tile([128, nt], F32, tag=f"nmr{tag}", bufs=2)
                nc.vector.tensor_scalar_mul(m[:], ssum[:], 1.0 / D)
                nc.vector.tensor_scalar_mul(qsum[:], qsum[:], 1.0 / D)
                nc.vector.tensor_tensor(var[:], m[:], m[:], ALU.mult)
                nc.vector.tensor_tensor(var[:], qsum[:], var[:], ALU.subtract)
                # std = sqrt(var + eps); rstd = 1/std (HW divide, accurate)
                nc.scalar.activation(std[:], var[:], AF.Sqrt, bias=eps[:])
                nc.vector.reciprocal(rstd[:], std[:])
                nc.vector.scalar_tensor_tensor(
                    nmr[:], m[:], -1.0, rstd[:], ALU.mult, ALU.mult)
                return rstd, nmr

            def ln_apply(dst_ap_fn, src, nt, rstd, nmr):
                for i in range(nt):
                    nc.scalar.activation(
                        dst_ap_fn(i), src[:, i, :], AF.Identity,
                        bias=nmr[:, i:i + 1], scale=rstd[:, i:i + 1])

            def transpose_block(src_ap, dst_ap, name):
                """bf16 [128,128] transpose via PE; dst gets src.T."""
                ps = pt_pool.tile([128, 128], BF16, tag="pt", name=name)
                nc.tensor.transpose(ps[:], src_ap, ident[:])
                nc.any.tensor_copy(dst_ap, ps[:])

            for l in range(nlayers):
                vb_cur, vt_cur = vbf[l % 2], vT[l % 2]
                vb_nxt, vt_nxt = vbf[(l + 1) % 2], vT[(l + 1) % 2]
                a_red = {}
                e_red = {}

                # ---------------- X + rope; G; a; AR(a) ----------------
                for tci in range(TCN):
                    t0c = tci * TCW
                    sl = slice(t0c, t0c + TCW)
                    for c in range(NPAIR):
                        x1 = chbf.tile([128, TCW], BF16, tag="x1", name=f"x1_{c}")
                        x2 = chbf.tile([128, TCW], BF16, tag="x2", name=f"x2_{c}")
                        x_chunk_mm(vt_cur, c, tci, x1)
                        x_chunk_mm(vt_cur, NPAIR + c, tci, x2)
                        co = trig.tile([128, TCW], BF16, tag="cos", name=f"co_{c}")
                        si = trig.tile([128, TCW], BF16, tag="sin", name=f"si_{c}")
                        nc.sync.dma_start(co[:], cs_ap[c, tci, :, :])
                        nc.sync.dma_start(si[:], sn_ap[c, tci, :, :])
                        p1 = chbf.tile([128, TCW], BF16, tag="p1", name=f"p1_{c}")
                        p2 = chbf.tile([128, TCW], BF16, tag="p2", name=f"p2_{c}")
                        # xr1 = x1*cos - x2*sin
                        nc.vector.tensor_tensor(p1[:], x1[:], co[:], ALU.mult)
                        nc.vector.tensor_tensor(p2[:], x2[:], si[:], ALU.mult)
                        nc.vector.tensor_tensor(
                            xrT[:, c, sl], p1[:], p2[:], ALU.subtract)
                        # xr2 = x2*cos + x1*sin
                        nc.vector.tensor_tensor(p1[:], x2[:], co[:], ALU.mult)
                        nc.vector.tensor_tensor(p2[:], x1[:], si[:], ALU.mult)
                        nc.vector.tensor_tensor(
                            xrT[:, NPAIR + c, sl], p1[:], p2[:], ALU.add)

                    # G blocks (strict upper in (s, t))
                    for st in range(4 * tci + 4):
                        tg0 = max(st * 128, t0c)
                        nw = t0c + TCW - tg0
                        pg = pg_pool.tile([128, TCW], F32, tag="pg", name=f"pg_{st}")
                        for cc in range(NCH):
                            nc.tensor.matmul(
                                pg[:, :nw],
                                xrT[:, cc, st * 128:(st + 1) * 128],
                                xrT[:, cc, tg0:tg0 + nw],
                                start=(cc == 0), stop=(cc == NCH - 1),
                            )
                        if tg0 == st * 128:
                            nc.vector.tensor_tensor(
                                S[:, st, tg0:tg0 + 128], pg[:, 0:128], maskd[:],
                                ALU.mult)
                            if nw > 128:
                                nc.vector.tensor_copy(
                                    S[:, st, tg0 + 128:tg0 + nw], pg[:, 128:nw])
                        else:
                            nc.vector.tensor_copy(S[:, st, tg0:tg0 + nw], pg[:, :nw])

                    # a partial
                    a_loc = s32.tile([128, 4, D], F32, tag="a_loc", bufs=2,
                                     name=f"a_loc_{l}_{tci}")
                    for i in range(4):
                        gt = 4 * tci + i
                        pa = pa_pool.tile([128, D], F32, tag="pa", name=f"pa_{gt}")
                        for st in range(gt + 1):
                            nc.tensor.matmul(
                                pa[:], S[:, st, gt * 128:(gt + 1) * 128],
                                vb_cur[:, st, :],
                                start=(st == 0), stop=(st == gt),
                            )
                        nc.any.tensor_copy(a_loc[:, i, :], pa[:])

                    ain = dram.tile([128, 4, D], F32, tag="ain", name=f"ain_{l}_{tci}")
                    aout = dram.tile([128, 4, D], F32, tag="aout",
                                     name=f"aout_{l}_{tci}")
                    nc.sync.dma_start(ain[:], a_loc[:])
                    nc.gpsimd.collective_compute(
                        "AllReduce", ALU.add, replica_groups=REPLICA_PAIRS,
                        ins=[ain.opt()], outs=[aout.opt()])
                    ar = s32.tile([128, 4, D], F32, tag="a_red", bufs=2,
                                  name=f"a_red_{l}_{tci}")
                    nc.sync.dma_start(ar[:], aout[:])
                    a_red[tci] = ar

                # ---------------- ln(a); Y + gate + E; AR(e) ----------------
                for tci in range(TCN):
                    ar = a_red[tci]
                    rstd, nmr = ln_stats(ar, 4, "a")
                    lnA = sbf.tile([128, 4, D], BF16, tag="lnA", bufs=1,
                                   name=f"lnA_{l}_{tci}")
                    ln_apply(lambda i: lnA[:, i, :], ar, 4, rstd, nmr)
                    lat = sbf.tile([128, 2, TCW], BF16, tag="lnAT", bufs=2,
                                   name=f"lnAT_{l}_{tci}")
                    for i in range(4):
                        for dc in range(2):
                            transpose_block(
                                lnA[:, i, dc * 128:(dc + 1) * 128],
                                lat[:, dc, i * 128:(i + 1) * 128],
                                f"ptA_{i}_{dc}")

                    pe0 = pe_pool.tile([128, TCW], F32, tag="pe", name=f"pe0_{tci}")
                    pe1 = pe_pool.tile([128, TCW], F32, tag="pe", name=f"pe1_{tci}")
                    for c in range(NCH):
                        py = pxy_pool.tile([128, TCW], F32, tag="pxy",
                                           name=f"py_{c}_{tci}")
                        for dc in range(2):
                            nc.tensor.matmul(
                                py[:], wy[:, dc, c * 128:(c + 1) * 128],
                                lat[:, dc, :],
                                start=(dc == 0), stop=(dc == 1),
                            )
                        xg = chbf.tile([128, TCW], BF16, tag="xg", name=f"xg_{c}")
                        x_chunk_mm(vt_cur, c, tci, xg)
                        yc = chbf.tile([128, TCW], BF16, tag="yc", name=f"yc_{c}")
                        nc.vector.scalar_tensor_tensor(
                            yc[:], py[:], 0.0, xg[:], ALU.max, ALU.mult)
                        ec = encp.tile([128, D], BF16, tag="enc", name=f"ec_{c}")
                        nc.sync.dma_start(ec[:], enc_ap[c, :, :])
                        for dc, pe in ((0, pe0), (1, pe1)):
                            nc.tensor.matmul(
                                pe[:], ec[:, dc * 128:(dc + 1) * 128], yc[:],
                                start=(c == 0), stop=(c == NCH - 1),
                            )
                    # evac e^T (f32), AllReduce in that layout (elementwise-safe),
                    # then cast bf16 and transpose to [t, d].
                    eT = s32.tile([128, 2, TCW], F32, tag="eT", bufs=1,
                                  name=f"eT_{l}_{tci}")
                    nc.any.tensor_copy(eT[:, 0, :], pe0[:])
                    nc.any.tensor_copy(eT[:, 1, :], pe1[:])
                    ein = dram.tile([128, 2, TCW], F32, tag="ein",
                                    name=f"ein_{l}_{tci}")
                    eout = dram.tile([128, 2, TCW], F32, tag="eout",
                                     name=f"eout_{l}_{tci}")
                    nc.sync.dma_start(ein[:], eT[:])
                    nc.gpsimd.collective_compute(
                        "AllReduce", ALU.add, replica_groups=REPLICA_ALL,
                        ins=[ein.opt()], outs=[eout.opt()])
                    ert = s32.tile([128, 2, TCW], F32, tag="ert", bufs=1,
                                   name=f"ert_{l}_{tci}")
                    nc.sync.dma_start(ert[:], eout[:])
                    ertb = sbf.tile([128, 2, TCW], BF16, tag="ertb", bufs=1,
                                    name=f"ertb_{l}_{tci}")
                    nc.vector.tensor_copy(ertb[:], ert[:])
                    er = sbf.tile([128, 4, D], BF16, tag="e_red", bufs=2,
                                  name=f"e_red_{l}_{tci}")
                    for i in range(4):
                        for dc in range(2):
                            transpose_block(
                                ertb[:, dc, i * 128:(i + 1) * 128],
                                er[:, i, dc * 128:(dc + 1) * 128],
                                f"ptE_{i}_{dc}")
                    e_red[tci] = er

                # ---------------- v update ----------------
                for tci in range(TCN):
                    er = e_red[tci]
                    rstd, nmr = ln_stats(er, 4, "e")
                    lnE = s32.tile([128, 4, D], F32, tag="lnE", bufs=1,
                                   name=f"lnE_{l}_{tci}")
                    ln_apply(lambda i: lnE[:, i, :], er, 4, rstd, nmr)
                    w = s32.tile([128, 4, D], F32, tag="wres", bufs=1,
                                 name=f"w_{l}_{tci}")
                    nc.vector.tensor_tensor(
                        w[:], vb_cur[:, 4 * tci:4 * tci + 4, :], lnE[:], ALU.add)
                    rstd2, nmr2 = ln_stats(w, 4, "v")
                    for i in range(4):
                        gt = 4 * tci + i
                        nc.scalar.activation(
                            vb_nxt[:, gt, :], w[:, i, :], AF.Identity,
                            bias=nmr2[:, i:i + 1], scale=rstd2[:, i:i + 1])
                        for dc in range(2):
                            transpose_block(
                                vb_nxt[:, gt, dc * 128:(dc + 1) * 128],
                                vt_nxt[:, dc, gt * 128:(gt + 1) * 128],
                                f"ptV_{gt}_{dc}")

            # ---------------- readout ----------------
            vt_fin = vT[nlayers % 2]
            for gt in range(TT):
                ps = pa_pool.tile([128, VOCAB], F32, tag="pa", name=f"pro_{gt}")
                for dc in range(2):
                    nc.tensor.matmul(
                        ps[:], vt_fin[:, dc, gt * 128:(gt + 1) * 128],
                        ro[:, dc, :],
                        start=(dc == 0), stop=(dc == 1),
                    )
                ob = s32.tile([128, VOCAB], F32, tag="outsb", bufs=2,
                              name=f"ob_{gt}")
                nc.any.tensor_copy(ob[:], ps[:])
                nc.sync.dma_start(out_d[gt * 128:(gt + 1) * 128, :], ob[:])

    nc.compile()
    return nc


def prep_inputs(inputs):
    """Full inputs -> per-core in_maps (host-side shard + precompute)."""
    bf = ml_dtypes.bfloat16
    idx = np.asarray(inputs["idx"], dtype=np.int32)
    wte = np.asarray(inputs["wte"], dtype=np.float32)
    enc = np.asarray(inputs["encoder"], dtype=np.float32)
    dx = np.asarray(inputs["decoder_x"], dtype=np.float32)
    dy = np.asarray(inputs["decoder_y"], dtype=np.float32)
    ro = np.asarray(inputs["readout"], dtype=np.float32)

    # embedding + initial layernorm (host)
    v0 = wte[idx[0]]
    m = v0.mean(-1, keepdims=True)
    va = v0.var(-1, keepdims=True)
    v0 = ((v0 - m) / np.sqrt(va + 1e-5)).astype(np.float32)  # [T, D]
    v0b = np.ascontiguousarray(
        v0.reshape(TT, 128, D).transpose(1, 0, 2)).astype(bf)
    v0t = np.ascontiguousarray(
        v0.T.reshape(2, 128, T).transpose(1, 0, 2)).astype(bf)

    half_g = NH // 2
    inv = 1.0 / (ROPE_BASE ** (np.arange(half_g, dtype=np.float32) / half_g))
    tarr = np.arange(T, dtype=np.float32)

    mask = np.triu(np.ones((128, 128), np.float32), k=1)  # keep s < t
    ident = np.eye(128, dtype=np.float32).astype(bf)
    ro_arr = np.ascontiguousarray(
        ro.reshape(2, 128, VOCAB).transpose(1, 0, 2)).astype(bf)

    in_maps = []
    for c in range(NCORES):
        h, p = c // 2, c % 2
        j0, j1 = p * HALF, (p + 1) * HALF
        cols = np.r_[j0:j1, half_g + j0:half_g + j1]
        wx_c = dx[h][:, cols]   # [256, 4096]
        wy_c = dy[h][:, cols]
        enc_c = enc[h * NH:(h + 1) * NH][cols]  # [4096, 256]

        wx_arr = np.ascontiguousarray(
            wx_c.reshape(2, 128, NLOC).transpose(1, 0, 2)).astype(bf)
        wy_arr = np.ascontiguousarray(
            wy_c.reshape(2, 128, NLOC).transpose(1, 0, 2)).astype(bf)
        enc_arr = np.ascontiguousarray(enc_c.reshape(NCH, 128, D)).astype(bf)

        ang = tarr[:, None] * inv[None, j0:j1]      # [T, 2048]
        cos = np.cos(ang).T.astype(np.float32)      # [2048, T]
        sin = np.sin(ang).T.astype(np.float32)
        cs_arr = np.ascontiguousarray(
            cos.reshape(NPAIR, 128, TCN, TCW).transpose(0, 2, 1, 3)).astype(bf)
        sn_arr = np.ascontiguousarray(
            sin.reshape(NPAIR, 128, TCN, TCW).transpose(0, 2, 1, 3)).astype(bf)

        in_maps.append({
            "wx": wx_arr, "wy": wy_arr, "enc": enc_arr,
            "cs": cs_arr, "sn": sn_arr, "ro": ro_arr,
            "v0b": v0b, "v0t": v0t, "maskd": mask, "ident": ident,
        })
    return in_maps


_NC_CACHE = {}


def get_nc(nlayers: int = L):
    if nlayers not in _NC_CACHE:
        _NC_CACHE[nlayers] = build(nlayers)
    return _NC_CACHE[nlayers]


def kernel(**inputs) -> np.ndarray:
    nc = get_nc()
    in_maps = prep_inputs(inputs)
    res = run_bass_kernel_spmd(nc, in_maps, core_ids=list(range(NCORES)))
    out = res.results[0]["out"].astype(np.float32)
    return out.reshape(1, T, VOCAB)
